# revision 22
# baseline (speedup 1.0000x reference)
"""Expert-parallel MoE (Kimi/DeepSeek-V3 style sparse block) on 8 trn2 NeuronCores.

Strategy (v2, bf16 balanced):
  - Host computes the sigmoid gate + group-limited top-2 routing in numpy
    float64 and gathers each expert's tokens into a fixed-capacity,
    transposed bf16 batch (capacity 1216 = 2 full 512-chunks + one 192
    remainder chunk).
  - Core e runs expert e's SwiGLU FFN over its gathered tokens (3 chunk
    jobs), plus one half (F=512 slice) of the shared-expert FFN over token
    slice [(e%4)*1024 : (e%4+1)*1024] (2 more 512-chunk jobs).  All matmuls
    run in bf16 (full-rate on the PE, fp32 PSUM accumulate), activations in
    fp32.
  - The down-projection of job j-1 is software-pipelined into the up-phase
    of job j so the PE never waits on the scalar/vector engines.
  - Host scatter-adds the weighted expert outputs and the two shared-half
    outputs back into the full [B,S,D] result.
"""

from contextlib import ExitStack

import numpy as np
import ml_dtypes

import concourse.bacc as bacc
import concourse.tile as tile
import concourse.mybir as mybir
from concourse import bass_utils

# --- model dims (hardcoded per problem spec) ---
B, S, D = 2, 2048, 1024
T = B * S                 # 4096 tokens
E, F = 8, 512             # routed experts / expert intermediate
SH = 1024                 # shared intermediate
TOP_K, N_GROUP, TOPK_GROUP = 2, 4, 2
SCALE = 2.5

N_CORES = 8
P = 128                   # SBUF partitions
NT = 512                  # full token chunk (matmul moving free dim)
KD = D // P               # 8 contraction tiles for D
KF = F // P               # 4 contraction tiles for F
SSL = 1024                # shared-expert tokens per core (half of F each)

F32 = mybir.dt.float32
BF = mybir.dt.bfloat16
BF_NP = ml_dtypes.bfloat16

_CACHE: dict = {}


def _emit(nc, REM):
    """Per-core program: 3 expert-chunk jobs + 2 shared-half-chunk jobs."""
    xe0 = nc.dram_tensor("xe0", [P, KD, NT], BF, kind="ExternalInput").ap()
    xe1 = nc.dram_tensor("xe1", [P, KD, NT], BF, kind="ExternalInput").ap()
    xer = nc.dram_tensor("xer", [P, KD, REM], BF, kind="ExternalInput").ap()
    xs0 = nc.dram_tensor("xs0", [P, KD, NT], BF, kind="ExternalInput").ap()
    xs1 = nc.dram_tensor("xs1", [P, KD, NT], BF, kind="ExternalInput").ap()
    we1 = nc.dram_tensor("we1", [P, KD, F], BF, kind="ExternalInput").ap()
    we3 = nc.dram_tensor("we3", [P, KD, F], BF, kind="ExternalInput").ap()
    we2 = nc.dram_tensor("we2", [P, KF, D], BF, kind="ExternalInput").ap()
    ws1 = nc.dram_tensor("ws1", [P, KD, F], BF, kind="ExternalInput").ap()
    ws3 = nc.dram_tensor("ws3", [P, KD, F], BF, kind="ExternalInput").ap()
    ws2 = nc.dram_tensor("ws2", [P, KF, D], BF, kind="ExternalInput").ap()
    ye0 = nc.dram_tensor("ye0", [P, KD, NT], BF, kind="ExternalOutput").ap()
    ye1 = nc.dram_tensor("ye1", [P, KD, NT], BF, kind="ExternalOutput").ap()
    yer = nc.dram_tensor("yer", [P, KD, REM], BF, kind="ExternalOutput").ap()
    ys0 = nc.dram_tensor("ys0", [P, KD, NT], BF, kind="ExternalOutput").ap()
    ys1 = nc.dram_tensor("ys1", [P, KD, NT], BF, kind="ExternalOutput").ap()

    silu = mybir.ActivationFunctionType.Silu

    with tile.TileContext(nc) as tc, ExitStack() as ctx:
        wpool = ctx.enter_context(tc.tile_pool(name="wpool", bufs=1))
        xpool = ctx.enter_context(tc.tile_pool(name="xpool", bufs=2))
        apool = ctx.enter_context(tc.tile_pool(name="apool", bufs=3))
        hpool = ctx.enter_context(tc.tile_pool(name="hpool", bufs=2))
        ypool = ctx.enter_context(tc.tile_pool(name="ypool", bufs=2))
        pspool = ctx.enter_context(tc.tile_pool(name="pspool", bufs=1, space="PSUM"))

        def load_x(src, n, eng=None):
            xsb = xpool.tile([P, KD, NT], BF, name="xsb", tag="x")
            (eng or nc.sync).dma_start(xsb[:, :, :n], src)
            return xsb

        # PE warmup: dummy matmuls fill the DMA-latency window (~7-11.5us)
        # so the HAM clock gate is at full rate when real data lands.
        wmw = wpool.tile([P, P], BF, name="wmw", tag="wmw")
        nc.vector.memset(wmw[:], 0)
        wmx = wpool.tile([P, NT], BF, name="wmx", tag="wmx")
        nc.vector.memset(wmx[:], 0)
        wmp = pspool.tile([P, NT], F32, name="warm", tag="dn0")
        for _ in range(10):
            nc.tensor.matmul(wmp[:], wmw[:], wmx[:], start=True, stop=True)

        # Critical path: job0's up-phase inputs (x0, we1, we3) split across
        # THREE parallel HWDGE rings (sync, scalar, vector) so their
        # completion latencies pipeline in parallel.  Job 0 runs all its w1
        # matmuls before any w3 matmul, so only x0+we1 gate the first
        # ~13us of PE work.
        x_first = xpool.tile([P, KD, NT], BF, name="xsb", tag="x")
        w1e_sb = wpool.tile([P, KD, F], BF, name="w1e", tag="w1e")
        w3e_sb = wpool.tile([P, KD, F], BF, name="w3e", tag="w3e")
        w2e_sb = wpool.tile([P, KF, D], BF, name="w2e", tag="w2e")
        nc.sync.dma_start(x_first[:, 0:4, :], xe0[:, 0:4])
        nc.gpsimd.dma_start(x_first[:, 4:KD, :], xe0[:, 4:KD])
        nc.scalar.dma_start(w1e_sb[:, 0:4], we1[:, 0:4])
        nc.scalar.dma_start(w1e_sb[:, 4:KD], we1[:, 4:KD])
        nc.sync.dma_start(w3e_sb[:, 0:4], we3[:, 0:4])
        nc.gpsimd.dma_start(w3e_sb[:, 4:KD], we3[:, 4:KD])
        nc.scalar.dma_start(w2e_sb[:], we2)
        wexp = (w1e_sb, w3e_sb, w2e_sb)
        w1s_sb = wpool.tile([P, KD, F], BF, name="w1s", tag="w1s")
        nc.scalar.dma_start(w1s_sb[:], ws1)
        w3s_sb = wpool.tile([P, KD, F], BF, name="w3s", tag="w3s")
        nc.scalar.dma_start(w3s_sb[:], ws3)
        w2s_sb = wpool.tile([P, KF, D], BF, name="w2s", tag="w2s")
        nc.scalar.dma_start(w2s_sb[:], ws2)
        wsh = (w1s_sb, w3s_sb, w2s_sb)

        jobs = [
            dict(x=xe0, y=ye0, n=NT, w=wexp),
            dict(x=xe1, y=ye1, n=NT, w=wexp),
            dict(x=xs0, y=ys0, n=NT, w=wsh),
            dict(x=xs1, y=ys1, n=NT, w=wsh),
            dict(x=xer, y=yer, n=REM, w=wexp),
        ]
        NJ = len(jobs)
        jobs[0]["xsb"] = x_first

        def down_group(j, md):
            """Down-proj for D-tile md of job j (needs job j's ht tiles)."""
            job = jobs[j]
            n = job["n"]
            w2sb = job["w"][2]
            yp = pspool.tile([P, NT], F32, name=f"dn{md % 3}", tag=f"dn{md % 3}")
            for kf in range(KF):
                nc.tensor.matmul(
                    yp[:, :n],
                    w2sb[:, kf, md * P:(md + 1) * P],
                    job["ht"][kf][:, :n],
                    start=(kf == 0), stop=(kf == KF - 1),
                )
            ysb = job["ysb"]
            if md % 2 == 0:
                nc.scalar.copy(ysb[:, md, :n], yp[:, :n])
            else:
                nc.vector.tensor_copy(ysb[:, md, :n], yp[:, :n])
            if job.get("final"):
                # fine-grained stores on both rings so the tail drains fast
                if md % 2 == 1:
                    eng = nc.scalar if md % 4 == 1 else nc.sync
                    eng.dma_start(job["y"][:, md - 1:md + 1],
                                  ysb[:, md - 1:md + 1, :n])
            elif md == KD // 2 - 1:
                nc.sync.dma_start(job["y"][:, 0:KD // 2],
                                  ysb[:, 0:KD // 2, :n])
            elif md == KD - 1:
                nc.sync.dma_start(job["y"][:, KD // 2:KD],
                                  ysb[:, KD // 2:KD, :n])

        def up_mm(ps, wsb, xsb, mf, n):
            for k in range(KD):
                nc.tensor.matmul(
                    ps[:, :n],
                    wsb[:, k, mf * P:(mf + 1) * P],
                    xsb[:, k, :n],
                    start=(k == 0), stop=(k == KD - 1),
                )

        for j, job in enumerate(jobs):
            n = job["n"]
            w1sb, w3sb, _ = job["w"]
            xsb = job.get("xsb")
            if xsb is None:
                xsb = load_x(job["x"], n)
            # prefetch next x while this job computes
            if j + 1 < NJ:
                jobs[j + 1]["xsb"] = load_x(jobs[j + 1]["x"], jobs[j + 1]["n"])
            job["ht"] = []
            job["ysb"] = ypool.tile([P, KD, NT], BF, name="ysb", tag="ysb")
            if j == 0:
                # h1-first: only we1+x0 gate the first half of the job
                avs = []
                for mf in range(KF):
                    h1 = pspool.tile([P, NT], F32, name=f"u{mf}", tag=f"u{mf}")
                    up_mm(h1, w1sb, xsb, mf, n)
                    a = apool.tile([P, NT], F32, name=f"a{mf}", tag=f"a{mf}")
                    nc.scalar.activation(a[:, :n], h1[:, :n], silu)
                    avs.append(a)
                for mf in range(KF):
                    tag = ("dn0", "dn1", "dn2", "u0")[mf]
                    h3 = pspool.tile([P, NT], F32, name=tag, tag=tag)
                    up_mm(h3, w3sb, xsb, mf, n)
                    ht = hpool.tile([P, NT], BF, name=f"ht{mf}", tag=f"ht{mf}")
                    nc.vector.tensor_mul(ht[:, :n], avs[mf][:, :n], h3[:, :n])
                    job["ht"].append(ht)
                continue
            for mf in range(KF):
                h1 = pspool.tile([P, NT], F32, name=f"u{2 * (mf % 2)}",
                                 tag=f"u{2 * (mf % 2)}")
                up_mm(h1, w1sb, xsb, mf, n)
                h3 = pspool.tile([P, NT], F32, name=f"u{2 * (mf % 2) + 1}",
                                 tag=f"u{2 * (mf % 2) + 1}")
                up_mm(h3, w3sb, xsb, mf, n)
                a = apool.tile([P, NT], F32, name="asb", tag="a")
                nc.scalar.activation(a[:, :n], h1[:, :n], silu)
                ht = hpool.tile([P, NT], BF, name=f"ht{mf}", tag=f"ht{mf}")
                nc.vector.tensor_mul(ht[:, :n], a[:, :n], h3[:, :n])
                job["ht"].append(ht)
                # interleave the previous job's down-projection
                down_group(j - 1, 2 * mf)
                down_group(j - 1, 2 * mf + 1)
            if j == NJ - 1:
                job["final"] = True
                for md in range(KD):
                    down_group(j, md)


def _get_nc(REM):
    key = ("nc", REM)
    if key not in _CACHE:
        nc = bacc.Bacc("TRN2", target_bir_lowering=False, debug=False,
                       num_devices=N_CORES)
        _emit(nc, REM)
        nc.compile()
        _CACHE[key] = nc
    return _CACHE[key]


def _gate_numpy(x2d):
    """Replicates reference _moe_gate in float64 (routing-stable)."""
    xl = x2d.astype(np.float64)
    logits = xl @ _CACHE["gw64"].T
    scores = 1.0 / (1.0 + np.exp(-logits))
    sc = scores + _CACHE["gb64"][None, :]
    grp = sc.reshape(T, N_GROUP, E // N_GROUP)
    group_scores = np.sort(grp, axis=-1)[:, :, -2:].sum(-1)
    gidx = np.argsort(-group_scores, axis=-1, kind="stable")[:, :TOPK_GROUP]
    gmask = np.zeros((T, N_GROUP), bool)
    gmask[np.arange(T)[:, None], gidx] = True
    smask = np.repeat(gmask, E // N_GROUP, axis=1)
    tmp = np.where(smask, sc, 0.0)
    tidx = np.argsort(-tmp, axis=-1, kind="stable")[:, :TOP_K]
    tw = np.take_along_axis(scores, tidx, axis=1)
    tw = tw / (tw.sum(-1, keepdims=True) + 1e-20)
    return tidx, (tw * SCALE).astype(np.float32)


def _ffn_host(x, w1e, w2e, w3e):
    """Host fallback for capacity-overflow tokens (rare)."""
    h = x @ w1e.T
    h = (h / (1.0 + np.exp(-h))) * (x @ w3e.T)
    return h @ w2e.T


def _prep_w13(w):
    """[F, D] expert up-proj weight -> [P, KD, F] bf16 stationary layout."""
    wt = np.ascontiguousarray(w.T)                       # [D, F]
    return np.ascontiguousarray(
        wt.reshape(KD, P, F).transpose(1, 0, 2).astype(BF_NP))


def _prep_w2(w):
    """[D, F] down-proj weight -> [P, KF, D] bf16 stationary layout."""
    wt = np.ascontiguousarray(w.T)                       # [F, D]
    return np.ascontiguousarray(
        wt.reshape(KF, P, D).transpose(1, 0, 2).astype(BF_NP))


def _chunks_from_cols(xg):
    """[D, n] token-column block -> ([P, KD, n] layout)."""
    n = xg.shape[1]
    return np.ascontiguousarray(xg.reshape(KD, P, n).transpose(1, 0, 2))


def _uncols(arr):
    """[P, KD, n] device layout -> [n, D] float32 token rows."""
    p, kd, n = arr.shape
    return arr.transpose(1, 0, 2).reshape(D, n).T.astype(np.float32)


def kernel(hidden_states, gate_w, gate_bias, w1, w2, w3,
           shared_gate_w, shared_up_w, shared_down_w):
    hidden_states = np.ascontiguousarray(np.asarray(hidden_states, np.float32))
    gate_w = np.asarray(gate_w, np.float32)
    gate_bias = np.asarray(gate_bias, np.float32)
    w1 = np.asarray(w1, np.float32)
    w2 = np.asarray(w2, np.float32)
    w3 = np.asarray(w3, np.float32)
    shared_gate_w = np.asarray(shared_gate_w, np.float32)
    shared_up_w = np.asarray(shared_up_w, np.float32)
    shared_down_w = np.asarray(shared_down_w, np.float32)

    _CACHE["gw64"] = gate_w.astype(np.float64)
    _CACHE["gb64"] = gate_bias.astype(np.float64)

    x2d = hidden_states.reshape(T, D)
    tidx, tw = _gate_numpy(x2d)

    # remainder-chunk size from the actual routing (recompile if it changes)
    max_n = int(np.bincount(tidx.ravel(), minlength=E).max())
    REM = min(NT, max(64, -(-max(max_n - 2 * NT, 1) // 32) * 32))
    CAPE = 2 * NT + REM

    x2dT_bf = np.ascontiguousarray(x2d.T).astype(BF_NP)  # [D, T]

    in_maps = []
    idx_list, wt_list, n_list = [], [], []
    overflow = []
    for e in range(E):
        rows, slots = np.nonzero(tidx == e)
        n = len(rows)
        if n > CAPE:
            overflow.append((e, rows[CAPE:], slots[CAPE:]))
            rows, slots = rows[:CAPE], slots[:CAPE]
            n = CAPE
        idx_list.append(rows)
        wt_list.append(tw[rows, slots])
        n_list.append(n)
        xg = np.zeros((D, CAPE), BF_NP)
        xg[:, :n] = x2dT_bf[:, rows]
        xe = _chunks_from_cols(xg)                       # [P, KD, CAPE]
        sl = slice((e % 4) * SSL, (e % 4 + 1) * SSL)
        xs = _chunks_from_cols(x2dT_bf[:, sl])           # [P, KD, SSL]
        half = e // 4
        hf = slice(half * F, (half + 1) * F)
        in_maps.append({
            "xe0": np.ascontiguousarray(xe[:, :, 0:NT]),
            "xe1": np.ascontiguousarray(xe[:, :, NT:2 * NT]),
            "xer": np.ascontiguousarray(xe[:, :, 2 * NT:CAPE]),
            "xs0": np.ascontiguousarray(xs[:, :, 0:NT]),
            "xs1": np.ascontiguousarray(xs[:, :, NT:2 * NT]),
            "we1": _prep_w13(w1[e]),
            "we3": _prep_w13(w3[e]),
            "we2": _prep_w2(w2[e]),
            "ws1": _prep_w13(shared_gate_w[hf, :]),
            "ws3": _prep_w13(shared_up_w[hf, :]),
            "ws2": _prep_w2(shared_down_w[:, hf]),
        })

    nc = _get_nc(REM)
    res = bass_utils.run_bass_kernel_spmd(
        nc, in_maps, core_ids=list(range(N_CORES))
    )
    _CACHE["last_res"] = res

    y = np.zeros((T, D), np.float32)
    for e in range(E):
        out = res.results[e]
        ye = np.concatenate(
            [np.asarray(out["ye0"], np.float32),
             np.asarray(out["ye1"], np.float32),
             np.asarray(out["yer"], np.float32)], axis=2)  # [P, KD, CAPE]
        n = n_list[e]
        if n:
            yrows = _uncols(ye[:, :, :])[:n]             # [n, D]
            y[idx_list[e]] += wt_list[e][:, None] * yrows
        ysh = np.concatenate(
            [np.asarray(out["ys0"], np.float32),
             np.asarray(out["ys1"], np.float32)], axis=2)  # [P, KD, SSL]
        sl = slice((e % 4) * SSL, (e % 4 + 1) * SSL)
        y[sl] += _uncols(ysh)
    for e, rows, slots in overflow:
        y[rows] += tw[rows, slots][:, None] * _ffn_host(
            x2d[rows], w1[e], w2[e], w3[e])

    return y.reshape(B, S, D)


# revision 26
# speedup vs baseline: 1.0369x; 1.0369x over previous
"""Expert-parallel MoE (Kimi/DeepSeek-V3 style sparse block) on 8 trn2 NeuronCores.

Strategy (v2, bf16 balanced):
  - Host computes the sigmoid gate + group-limited top-2 routing in numpy
    float64 and gathers each expert's tokens into a fixed-capacity,
    transposed bf16 batch (capacity 1216 = 2 full 512-chunks + one 192
    remainder chunk).
  - Core e runs expert e's SwiGLU FFN over its gathered tokens (3 chunk
    jobs), plus one half (F=512 slice) of the shared-expert FFN over token
    slice [(e%4)*1024 : (e%4+1)*1024] (2 more 512-chunk jobs).  All matmuls
    run in bf16 (full-rate on the PE, fp32 PSUM accumulate), activations in
    fp32.
  - The down-projection of job j-1 is software-pipelined into the up-phase
    of job j so the PE never waits on the scalar/vector engines.
  - Host scatter-adds the weighted expert outputs and the two shared-half
    outputs back into the full [B,S,D] result.
"""

from contextlib import ExitStack

import numpy as np
import ml_dtypes

import concourse.bacc as bacc
import concourse.tile as tile
import concourse.mybir as mybir
from concourse import bass_utils

# --- model dims (hardcoded per problem spec) ---
B, S, D = 2, 2048, 1024
T = B * S                 # 4096 tokens
E, F = 8, 512             # routed experts / expert intermediate
SH = 1024                 # shared intermediate
TOP_K, N_GROUP, TOPK_GROUP = 2, 4, 2
SCALE = 2.5

N_CORES = 8
P = 128                   # SBUF partitions
NT = 512                  # full token chunk (matmul moving free dim)
KD = D // P               # 8 contraction tiles for D
KF = F // P               # 4 contraction tiles for F
SSL = 1024                # shared-expert tokens per core (half of F each)

F32 = mybir.dt.float32
BF = mybir.dt.bfloat16
BF_NP = ml_dtypes.bfloat16

_CACHE: dict = {}


def _emit(nc, REM):
    """Per-core program: 3 expert-chunk jobs + 2 shared-half-chunk jobs."""
    xe0 = nc.dram_tensor("xe0", [P, KD, NT], BF, kind="ExternalInput").ap()
    xe1 = nc.dram_tensor("xe1", [P, KD, NT], BF, kind="ExternalInput").ap()
    xer = nc.dram_tensor("xer", [P, KD, REM], BF, kind="ExternalInput").ap()
    xs0 = nc.dram_tensor("xs0", [P, KD, NT], BF, kind="ExternalInput").ap()
    xs1 = nc.dram_tensor("xs1", [P, KD, NT], BF, kind="ExternalInput").ap()
    # up-proj weights are mf-major: [P, KF, KD, 128]
    we1 = nc.dram_tensor("we1", [P, KF, KD, P], BF, kind="ExternalInput").ap()
    we3 = nc.dram_tensor("we3", [P, KF, KD, P], BF, kind="ExternalInput").ap()
    we2 = nc.dram_tensor("we2", [P, KF, D], BF, kind="ExternalInput").ap()
    ws1 = nc.dram_tensor("ws1", [P, KF, KD, P], BF, kind="ExternalInput").ap()
    ws3 = nc.dram_tensor("ws3", [P, KF, KD, P], BF, kind="ExternalInput").ap()
    ws2 = nc.dram_tensor("ws2", [P, KF, D], BF, kind="ExternalInput").ap()
    ye0 = nc.dram_tensor("ye0", [P, KD, NT], BF, kind="ExternalOutput").ap()
    ye1 = nc.dram_tensor("ye1", [P, KD, NT], BF, kind="ExternalOutput").ap()
    yer = nc.dram_tensor("yer", [P, KD, REM], BF, kind="ExternalOutput").ap()
    ys0 = nc.dram_tensor("ys0", [P, KD, NT], BF, kind="ExternalOutput").ap()
    ys1 = nc.dram_tensor("ys1", [P, KD, NT], BF, kind="ExternalOutput").ap()

    silu = mybir.ActivationFunctionType.Silu

    with tile.TileContext(nc) as tc, ExitStack() as ctx:
        wpool = ctx.enter_context(tc.tile_pool(name="wpool", bufs=1))
        xpool = ctx.enter_context(tc.tile_pool(name="xpool", bufs=2))
        apool = ctx.enter_context(tc.tile_pool(name="apool", bufs=3))
        hpool = ctx.enter_context(tc.tile_pool(name="hpool", bufs=2))
        ypool = ctx.enter_context(tc.tile_pool(name="ypool", bufs=2))
        pspool = ctx.enter_context(tc.tile_pool(name="pspool", bufs=1, space="PSUM"))

        def load_x(src, n, eng=None):
            xsb = xpool.tile([P, KD, NT], BF, name="xsb", tag="x")
            (eng or nc.sync).dma_start(xsb[:, :, :n], src)
            return xsb

        # PE warmup: dummy matmuls fill the DMA-latency window (~7-11.5us)
        # so the HAM clock gate is at full rate when real data lands.
        wmw = wpool.tile([P, P], BF, name="wmw", tag="wmw")
        nc.vector.memset(wmw[:], 0)
        wmx = wpool.tile([P, NT], BF, name="wmx", tag="wmx")
        nc.vector.memset(wmx[:], 0)
        wmp = pspool.tile([P, NT], F32, name="warm", tag="dn0")
        for _ in range(8):
            nc.tensor.matmul(wmp[:], wmw[:], wmx[:], start=True, stop=True)

        # Critical path: x0 split across both HWDGE rings; we1/we3 loaded
        # in per-mf 256KB pieces alternating rings so the pieces' DMA
        # completion latencies pipeline.  Job 0 runs all its w1 matmuls
        # before any w3 matmul, so only x0+we1 gate the first ~13us of PE
        # work, and each h1(mf) only needs its own we1 piece.
        x_first = xpool.tile([P, KD, NT], BF, name="xsb", tag="x")
        w1e_sb = wpool.tile([P, KF, KD, P], BF, name="w1e", tag="w1e")
        w3e_sb = wpool.tile([P, KF, KD, P], BF, name="w3e", tag="w3e")
        w2e_sb = wpool.tile([P, KF, D], BF, name="w2e", tag="w2e")
        nc.sync.dma_start(x_first[:, 0:4, :], xe0[:, 0:4])
        nc.scalar.dma_start(x_first[:, 4:KD, :], xe0[:, 4:KD])
        for mf in range(KF):
            eng = nc.sync if mf % 2 == 0 else nc.scalar
            eng.dma_start(w1e_sb[:, mf], we1[:, mf])
        for mf in range(KF):
            eng = nc.sync if mf % 2 == 0 else nc.scalar
            eng.dma_start(w3e_sb[:, mf], we3[:, mf])
        nc.scalar.dma_start(w2e_sb[:], we2)
        wexp = (w1e_sb, w3e_sb, w2e_sb)
        w1s_sb = wpool.tile([P, KF, KD, P], BF, name="w1s", tag="w1s")
        nc.scalar.dma_start(w1s_sb[:], ws1)
        w3s_sb = wpool.tile([P, KF, KD, P], BF, name="w3s", tag="w3s")
        nc.scalar.dma_start(w3s_sb[:], ws3)
        w2s_sb = wpool.tile([P, KF, D], BF, name="w2s", tag="w2s")
        nc.scalar.dma_start(w2s_sb[:], ws2)
        wsh = (w1s_sb, w3s_sb, w2s_sb)

        jobs = [
            dict(x=xe0, y=ye0, n=NT, w=wexp),
            dict(x=xe1, y=ye1, n=NT, w=wexp),
            dict(x=xs0, y=ys0, n=NT, w=wsh),
            dict(x=xs1, y=ys1, n=NT, w=wsh),
            dict(x=xer, y=yer, n=REM, w=wexp),
        ]
        NJ = len(jobs)
        jobs[0]["xsb"] = x_first

        def down_group(j, md):
            """Down-proj for D-tile md of job j (needs job j's ht tiles)."""
            job = jobs[j]
            n = job["n"]
            w2sb = job["w"][2]
            yp = pspool.tile([P, NT], F32, name=f"dn{md % 3}", tag=f"dn{md % 3}")
            for kf in range(KF):
                nc.tensor.matmul(
                    yp[:, :n],
                    w2sb[:, kf, md * P:(md + 1) * P],
                    job["ht"][kf][:, :n],
                    start=(kf == 0), stop=(kf == KF - 1),
                )
            ysb = job["ysb"]
            if md % 2 == 0:
                nc.scalar.copy(ysb[:, md, :n], yp[:, :n])
            else:
                nc.vector.tensor_copy(ysb[:, md, :n], yp[:, :n])
            if job.get("final"):
                # fine-grained stores on both rings so the tail drains fast
                if md % 2 == 1:
                    eng = nc.scalar if md % 4 == 1 else nc.sync
                    eng.dma_start(job["y"][:, md - 1:md + 1],
                                  ysb[:, md - 1:md + 1, :n])
            elif md == KD // 2 - 1:
                nc.sync.dma_start(job["y"][:, 0:KD // 2],
                                  ysb[:, 0:KD // 2, :n])
            elif md == KD - 1:
                nc.sync.dma_start(job["y"][:, KD // 2:KD],
                                  ysb[:, KD // 2:KD, :n])

        def up_mm(ps, wsb, xsb, mf, n):
            for k in range(KD):
                nc.tensor.matmul(
                    ps[:, :n],
                    wsb[:, mf, k, :],
                    xsb[:, k, :n],
                    start=(k == 0), stop=(k == KD - 1),
                )

        for j, job in enumerate(jobs):
            n = job["n"]
            w1sb, w3sb, _ = job["w"]
            xsb = job.get("xsb")
            if xsb is None:
                xsb = load_x(job["x"], n)
            # prefetch next x while this job computes
            if j + 1 < NJ:
                jobs[j + 1]["xsb"] = load_x(jobs[j + 1]["x"], jobs[j + 1]["n"])
            job["ht"] = []
            job["ysb"] = ypool.tile([P, KD, NT], BF, name="ysb", tag="ysb")
            if j == 0:
                # h1-first: only we1+x0 gate the first half of the job
                avs = []
                for mf in range(KF):
                    h1 = pspool.tile([P, NT], F32, name=f"u{mf}", tag=f"u{mf}")
                    up_mm(h1, w1sb, xsb, mf, n)
                    a = apool.tile([P, NT], F32, name=f"a{mf}", tag=f"a{mf}")
                    nc.scalar.activation(a[:, :n], h1[:, :n], silu)
                    avs.append(a)
                for mf in range(KF):
                    tag = ("dn0", "dn1", "dn2", "u0")[mf]
                    h3 = pspool.tile([P, NT], F32, name=tag, tag=tag)
                    up_mm(h3, w3sb, xsb, mf, n)
                    ht = hpool.tile([P, NT], BF, name=f"ht{mf}", tag=f"ht{mf}")
                    nc.vector.tensor_mul(ht[:, :n], avs[mf][:, :n], h3[:, :n])
                    job["ht"].append(ht)
                continue
            for mf in range(KF):
                h1 = pspool.tile([P, NT], F32, name=f"u{2 * (mf % 2)}",
                                 tag=f"u{2 * (mf % 2)}")
                up_mm(h1, w1sb, xsb, mf, n)
                h3 = pspool.tile([P, NT], F32, name=f"u{2 * (mf % 2) + 1}",
                                 tag=f"u{2 * (mf % 2) + 1}")
                up_mm(h3, w3sb, xsb, mf, n)
                a = apool.tile([P, NT], F32, name="asb", tag="a")
                nc.scalar.activation(a[:, :n], h1[:, :n], silu)
                ht = hpool.tile([P, NT], BF, name=f"ht{mf}", tag=f"ht{mf}")
                nc.vector.tensor_mul(ht[:, :n], a[:, :n], h3[:, :n])
                job["ht"].append(ht)
                # interleave the previous job's down-projection
                down_group(j - 1, 2 * mf)
                down_group(j - 1, 2 * mf + 1)
            if j == NJ - 1:
                job["final"] = True
                for md in range(KD):
                    down_group(j, md)


def _get_nc(REM):
    key = ("nc", REM)
    if key not in _CACHE:
        nc = bacc.Bacc("TRN2", target_bir_lowering=False, debug=False,
                       num_devices=N_CORES)
        _emit(nc, REM)
        nc.compile()
        _CACHE[key] = nc
    return _CACHE[key]


def _gate_numpy(x2d):
    """Replicates reference _moe_gate in float64 (routing-stable)."""
    xl = x2d.astype(np.float64)
    logits = xl @ _CACHE["gw64"].T
    scores = 1.0 / (1.0 + np.exp(-logits))
    sc = scores + _CACHE["gb64"][None, :]
    grp = sc.reshape(T, N_GROUP, E // N_GROUP)
    group_scores = np.sort(grp, axis=-1)[:, :, -2:].sum(-1)
    gidx = np.argsort(-group_scores, axis=-1, kind="stable")[:, :TOPK_GROUP]
    gmask = np.zeros((T, N_GROUP), bool)
    gmask[np.arange(T)[:, None], gidx] = True
    smask = np.repeat(gmask, E // N_GROUP, axis=1)
    tmp = np.where(smask, sc, 0.0)
    tidx = np.argsort(-tmp, axis=-1, kind="stable")[:, :TOP_K]
    tw = np.take_along_axis(scores, tidx, axis=1)
    tw = tw / (tw.sum(-1, keepdims=True) + 1e-20)
    return tidx, (tw * SCALE).astype(np.float32)


def _ffn_host(x, w1e, w2e, w3e):
    """Host fallback for capacity-overflow tokens (rare)."""
    h = x @ w1e.T
    h = (h / (1.0 + np.exp(-h))) * (x @ w3e.T)
    return h @ w2e.T


def _prep_w13(w):
    """[F, D] up-proj weight -> [P, KF, KD, 128] bf16 mf-major layout."""
    wt = np.ascontiguousarray(w.T)                       # [D, F]
    return np.ascontiguousarray(
        wt.reshape(KD, P, KF, P).transpose(1, 2, 0, 3).astype(BF_NP))


def _prep_w2(w):
    """[D, F] down-proj weight -> [P, KF, D] bf16 stationary layout."""
    wt = np.ascontiguousarray(w.T)                       # [F, D]
    return np.ascontiguousarray(
        wt.reshape(KF, P, D).transpose(1, 0, 2).astype(BF_NP))


def _chunks_from_cols(xg):
    """[D, n] token-column block -> ([P, KD, n] layout)."""
    n = xg.shape[1]
    return np.ascontiguousarray(xg.reshape(KD, P, n).transpose(1, 0, 2))


def _uncols(arr):
    """[P, KD, n] device layout -> [n, D] float32 token rows."""
    p, kd, n = arr.shape
    return arr.transpose(1, 0, 2).reshape(D, n).T.astype(np.float32)


def kernel(hidden_states, gate_w, gate_bias, w1, w2, w3,
           shared_gate_w, shared_up_w, shared_down_w):
    hidden_states = np.ascontiguousarray(np.asarray(hidden_states, np.float32))
    gate_w = np.asarray(gate_w, np.float32)
    gate_bias = np.asarray(gate_bias, np.float32)
    w1 = np.asarray(w1, np.float32)
    w2 = np.asarray(w2, np.float32)
    w3 = np.asarray(w3, np.float32)
    shared_gate_w = np.asarray(shared_gate_w, np.float32)
    shared_up_w = np.asarray(shared_up_w, np.float32)
    shared_down_w = np.asarray(shared_down_w, np.float32)

    _CACHE["gw64"] = gate_w.astype(np.float64)
    _CACHE["gb64"] = gate_bias.astype(np.float64)

    x2d = hidden_states.reshape(T, D)
    tidx, tw = _gate_numpy(x2d)

    # remainder-chunk size from the actual routing (recompile if it changes)
    max_n = int(np.bincount(tidx.ravel(), minlength=E).max())
    REM = min(NT, max(64, -(-max(max_n - 2 * NT, 1) // 32) * 32))
    CAPE = 2 * NT + REM

    x2dT_bf = np.ascontiguousarray(x2d.T).astype(BF_NP)  # [D, T]

    in_maps = []
    idx_list, wt_list, n_list = [], [], []
    overflow = []
    for e in range(E):
        rows, slots = np.nonzero(tidx == e)
        n = len(rows)
        if n > CAPE:
            overflow.append((e, rows[CAPE:], slots[CAPE:]))
            rows, slots = rows[:CAPE], slots[:CAPE]
            n = CAPE
        idx_list.append(rows)
        wt_list.append(tw[rows, slots])
        n_list.append(n)
        xg = np.zeros((D, CAPE), BF_NP)
        xg[:, :n] = x2dT_bf[:, rows]
        xe = _chunks_from_cols(xg)                       # [P, KD, CAPE]
        sl = slice((e % 4) * SSL, (e % 4 + 1) * SSL)
        xs = _chunks_from_cols(x2dT_bf[:, sl])           # [P, KD, SSL]
        half = e // 4
        hf = slice(half * F, (half + 1) * F)
        in_maps.append({
            "xe0": np.ascontiguousarray(xe[:, :, 0:NT]),
            "xe1": np.ascontiguousarray(xe[:, :, NT:2 * NT]),
            "xer": np.ascontiguousarray(xe[:, :, 2 * NT:CAPE]),
            "xs0": np.ascontiguousarray(xs[:, :, 0:NT]),
            "xs1": np.ascontiguousarray(xs[:, :, NT:2 * NT]),
            "we1": _prep_w13(w1[e]),
            "we3": _prep_w13(w3[e]),
            "we2": _prep_w2(w2[e]),
            "ws1": _prep_w13(shared_gate_w[hf, :]),
            "ws3": _prep_w13(shared_up_w[hf, :]),
            "ws2": _prep_w2(shared_down_w[:, hf]),
        })

    nc = _get_nc(REM)
    res = bass_utils.run_bass_kernel_spmd(
        nc, in_maps, core_ids=list(range(N_CORES))
    )
    _CACHE["last_res"] = res

    y = np.zeros((T, D), np.float32)
    for e in range(E):
        out = res.results[e]
        ye = np.concatenate(
            [np.asarray(out["ye0"], np.float32),
             np.asarray(out["ye1"], np.float32),
             np.asarray(out["yer"], np.float32)], axis=2)  # [P, KD, CAPE]
        n = n_list[e]
        if n:
            yrows = _uncols(ye[:, :, :])[:n]             # [n, D]
            y[idx_list[e]] += wt_list[e][:, None] * yrows
        ysh = np.concatenate(
            [np.asarray(out["ys0"], np.float32),
             np.asarray(out["ys1"], np.float32)], axis=2)  # [P, KD, SSL]
        sl = slice((e % 4) * SSL, (e % 4 + 1) * SSL)
        y[sl] += _uncols(ysh)
    for e, rows, slots in overflow:
        y[rows] += tw[rows, slots][:, None] * _ffn_host(
            x2d[rows], w1[e], w2[e], w3[e])

    return y.reshape(B, S, D)
